# revision 1
# baseline (speedup 1.0000x reference)
"""GATv2 message-passing kernel for 8 Trainium2 NeuronCores (Bass/Tile).

Strategy (per sharding hint): edges sharded contiguously across 8 cores,
node features replicated. Each core:
  1. projects all nodes -> DRAM table [NPAD, 64] (PE matmuls, lhsT = host-
     transposed node features so the contraction dim sits on partitions)
  2. streams its edge shard in chunks: 128-row indirect-DMA gathers of
     send/recv projections, PE edge-feature projection, ACT Mish, DVE
     logits + exp (max-free softmax: logits are bounded ~|10| for this
     model, exp stays in f32 range)
  3. scatter-adds per-edge payload [w*send(64); w(8)] into NT accumulator
     tables via CCE-add indirect DMAs. Intra-128 duplicate receivers are
     pre-summed with a PE selection-matrix matmul; only the first
     occurrence keeps the real row index (dups -> trash row), so every
     scatter DMA has unique indices; cross-DMA ordering per table is
     serialized by Tile's WAW tracking.
  4. merges tables, ReduceScatter over the 8 cores, divides numerator by
     denominator, writes its node-range output shard.
Host assembles the 8 shards.
"""
import sys
import os

sys.path.insert(0, "/opt/trn_rl_repo")
import numpy as np
import concourse.bass as bass
import concourse.mybir as mybir
import concourse.tile as tile
import concourse.tile as tile_mod
from concourse.masks import make_identity
from concourse.vector_clock import ScopedClock

# ---------------------------------------------------------------------------
# Environment workarounds (inlined so kernel.py is self-contained):
# 1. This walrus build only accepts ~1 sem-wait per TPB_CTRL instruction but
#    Tile piles every outstanding sem wait onto one SP drain at context exit.
#    Patch the drain to spread waits over nop carriers, and post-process all
#    instructions the same way.
# 2. Register the NEFF-emitting lowering for the 'axon' platform so 8-core
#    shard_map programs hit hardware instead of the CPU MultiCoreSim fallback.
# ---------------------------------------------------------------------------
try:
    from jax.interpreters import mlir as _mlir
    from concourse.bass2jax import (
        _bass_exec_p as _bep,
        _bass_exec_neuron_lowering as _benl,
        _partition_id_p as _pip,
        _partition_id_lowering as _pil,
    )

    _mlir.register_lowering(_bep, _benl, platform="axon")
    _mlir.register_lowering(_pip, _pil, platform="axon")
except Exception:  # pragma: no cover
    pass

_N_CARRIERS = 24


def _patched_drain_and_barrier(self, tick_clock, wait_clock):
    nc = self.nc
    nops = [nc.sync.nop(nofuse=True) for _ in range(_N_CARRIERS)]
    drain_inst = nc.sync.drain()
    wait_clock.add_sem_waits(
        drain_inst.ins, ScopedClock({None: tick_clock.global_clock}))
    waits = list(drain_inst.ins.sync_info.on_wait or [])
    if len(waits) > 1:
        assert len(waits) - 1 <= _N_CARRIERS
        drain_inst.ins.sync_info.on_wait = waits[:1]
        for nop, w in zip(nops, waits[1:]):
            si = nop.ins.sync_info
            if si is None:
                nop.ins.sync_info = mybir.SyncInfo(on_wait=[w], on_update=[])
            else:
                si.on_wait = [w]
    nc.all_engine_barrier()
    assert self.sems is not None
    popped = nc._tile_sem_poison_stack.pop()
    assert popped is self._sem_poison
    nc.clear_and_free_semaphores(list(self.sems.allocated().values()))
    nc.all_engine_barrier()


tile_mod.TileContext._drain_and_barrier = _patched_drain_and_barrier


def _split_excess_waits(nc, max_waits=1):
    for bbname, body in nc.bb_map.items():
        bb = body.bb
        insts = list(bb.instructions)
        out = []
        changed = False
        for ins in insts:
            si = ins.sync_info
            waits = list(si.on_wait) if si and si.on_wait else []
            if len(waits) > max_waits:
                keep = waits[:max_waits - 1] + [waits[-1]]
                extra = waits[max_waits - 1:-1]
                for w in extra:
                    nop = mybir.InstNoOp(
                        name=nc.get_next_instruction_name(), ins=[], outs=[])
                    nop.engine = ins.engine
                    nop.sync_info = mybir.SyncInfo(on_wait=[w], on_update=[])
                    nc.register_instruction(nop, overwrite=True)
                    out.append(nop)
                ins.sync_info.on_wait = keep
                changed = True
            out.append(ins)
        if changed:
            bb.instructions = out

F32 = mybir.dt.float32
I32 = mybir.dt.int32

N_NODES = 50000
N_EDGES = 1200000
IN_DIM = 128
EDGE_DIM = 64
EMBED = 64
HEADS = 8
PAY = EMBED + HEADS  # 72

N_CORES = 8
EPC = N_EDGES // N_CORES  # 150000
CHUNK = 2048
NCH = (EPC + CHUNK - 1) // CHUNK  # 74
NPAD = 50176  # 392*128
TRASH = 50100
PADNODE = 50150
NT = 8  # accumulator tables
NQ = 1  # SWDGE queues
RSROWS = NPAD // N_CORES  # 6272
RSC = RSROWS // 128  # 49


def _ap3(ap, mid_n):
    """[128, D] AP -> [128, mid_n(step0), D] broadcast view."""
    return bass.AP(ap.tensor, ap.offset, [ap.ap[0], [0, mid_n]] + list(ap.ap[1:]))


def _inner_b(ap, n):
    """Append a step-0 innermost free dim of size n (broadcast view)."""
    return bass.AP(ap.tensor, ap.offset, list(ap.ap) + [[0, n]])


def _q(bi, q):
    if q:
        bi.ins.queue = f"qPoolDynamic{q}"
    return bi


def build_nc(nch=NCH):
    slots = nch * CHUNK
    cols = slots // 128
    nc = bass.Bass(num_swdge_queues=NQ)

    nfT = nc.declare_dram_parameter("nfT", [IN_DIM, NPAD], F32, isOutput=False)
    eftT = nc.declare_dram_parameter("eftT", [EDGE_DIM, slots], F32, isOutput=False)
    s_wrap = nc.declare_dram_parameter("s_wrap", [128, cols], I32, isOutput=False)
    r_wrap = nc.declare_dram_parameter("r_wrap", [128, cols], I32, isOutput=False)
    W_e = nc.declare_dram_parameter("W", [IN_DIM, EMBED], F32, isOutput=False)
    Wb_e = nc.declare_dram_parameter("Wb", [128, EMBED], F32, isOutput=False)
    We_e = nc.declare_dram_parameter("We", [EDGE_DIM, EMBED], F32, isOutput=False)
    Web_e = nc.declare_dram_parameter("Web", [128, EMBED], F32, isOutput=False)
    a_e = nc.declare_dram_parameter("a16", [128, 16 * EMBED], F32, isOutput=False)
    lmask_e = nc.declare_dram_parameter("lmask", [128, 128], F32, isOutput=False)
    out_e = nc.declare_dram_parameter("out_shard", [128, RSC, EMBED], F32, isOutput=True)

    table = nc.dram_tensor("ntable", [NPAD, EMBED], F32)
    accs = [nc.dram_tensor(f"acc{t}", [NPAD, PAY], F32) for t in range(NT)]
    merged = nc.dram_tensor("merged", [NPAD, PAY], F32)
    rs_out = nc.dram_tensor("rs_out", [RSROWS, PAY], F32)

    with tile.TileContext(nc) as tc:
        with (
            tc.tile_pool(name="const", bufs=1) as cpool,
            tc.tile_pool(name="nproj", bufs=3) as npool,
            tc.tile_pool(name="mrg", bufs=2) as mpool,
            tc.tile_pool(name="fine", bufs=1) as fpool,
            tc.tile_pool(name="edgea", bufs=3) as apool,
            tc.tile_pool(name="edgeb", bufs=2) as bpool,
            tc.tile_pool(name="small", bufs=2) as spool,
            tc.tile_pool(name="zeros", bufs=1) as zpool,
            tc.tile_pool(name="ps_e", bufs=1, space="PSUM") as ps_e,
            tc.tile_pool(name="ps_t", bufs=1, space="PSUM") as ps_t,
            tc.tile_pool(name="ps_r", bufs=1, space="PSUM") as ps_r,
            tc.tile_pool(name="ps_p", bufs=1, space="PSUM") as ps_p,
        ):
            # ---- constants
            W_t = cpool.tile([IN_DIM, EMBED], F32)
            nc.sync.dma_start(out=W_t[:], in_=W_e[:])
            Wb_t = cpool.tile([128, EMBED], F32)
            nc.sync.dma_start(out=Wb_t[:], in_=Wb_e[:])
            We_t = cpool.tile([EDGE_DIM, EMBED], F32)
            nc.sync.dma_start(out=We_t[:], in_=We_e[:])
            Web_t = cpool.tile([128, EMBED], F32)
            nc.sync.dma_start(out=Web_t[:], in_=Web_e[:])
            a_t = cpool.tile([128, 16 * EMBED], F32)
            nc.sync.dma_start(out=a_t[:], in_=a_e[:])
            lm_t = cpool.tile([128, 128], F32)
            nc.sync.dma_start(out=lm_t[:], in_=lmask_e[:])
            idt = cpool.tile([128, 128], F32)
            make_identity(nc, idt[:])
            ones_t = cpool.tile([128, 1], F32)
            nc.gpsimd.memset(ones_t[:], 1.0)
            zero1_t = cpool.tile([128, 16], F32)
            nc.gpsimd.memset(zero1_t[:], 0.0)
            trash_t = cpool.tile([128, 16], I32)
            nc.gpsimd.memset(trash_t[:], TRASH)

            # ---- zero the accumulator tables (big dense writes)
            zt = zpool.tile([128, 2048], F32)
            nc.gpsimd.memset(zt[:], 0.0)
            zflat_cols = NPAD * PAY // 128  # 28224
            for t in range(NT):
                flat = accs[t][:].rearrange("n d -> (n d)").rearrange(
                    "(p c) -> p c", p=128)
                c0 = 0
                while c0 < zflat_cols:
                    cw = min(2048, zflat_cols - c0)
                    nc.sync.dma_start(out=flat[:, c0:c0 + cw], in_=zt[:, :cw])
                    c0 += cw

            # ---- phase 1: node projection -> table
            for t in range(NPAD // 128):
                nf_t = npool.tile([IN_DIM, 128], F32, tag="nf")
                nc.sync.dma_start(out=nf_t[:], in_=nfT[:, t * 128:(t + 1) * 128])
                ps = ps_e.tile([128, 16, EMBED], F32, space="PSUM", tag="ep")
                nc.tensor.matmul(out=ps[:, 0, :], lhsT=nf_t[:], rhs=W_t[:],
                                 start=True, stop=True)
                nb = npool.tile([128, EMBED], F32, tag="nb")
                nc.vector.tensor_add(nb[:], ps[:, 0, :], Wb_t[:])
                nc.sync.dma_start(out=table[t * 128:(t + 1) * 128, :], in_=nb[:])

            # ---- phase 2: edge chunks, software-pipelined.
            # Emission order interleaves stage A (loads + gathers) of chunk
            # k+2 BEFORE stage B (compute + dedup + scatters) of chunk k, so
            # the POOL engine's instruction stream never has a scatter whose
            # compute inputs are not already in flight ~2 chunks ahead.
            LOOKAHEAD = 2

            def stage_a(ch):
                col0 = ch * 16
                s_t = apool.tile([128, 16], I32, tag="sidx")
                nc.sync.dma_start(out=s_t[:], in_=s_wrap[:, col0:col0 + 16])
                r_t = apool.tile([128, 16], I32, tag="ridx")
                nc.sync.dma_start(out=r_t[:], in_=r_wrap[:, col0:col0 + 16])
                ef_t = apool.tile([EDGE_DIM, CHUNK], F32, tag="eft")
                nc.sync.dma_start(
                    out=ef_t[:], in_=eftT[:, ch * CHUNK:(ch + 1) * CHUNK])
                gs = apool.tile([128, 16, EMBED], F32, tag="gs")
                gr = apool.tile([128, 16, EMBED], F32, tag="gr")
                for c in range(16):
                    _q(nc.gpsimd.indirect_dma_start(
                        out=gs[:, c, :], out_offset=None, in_=table[:],
                        in_offset=bass.IndirectOffsetOnAxis(ap=s_t[:, c:c + 1], axis=0),
                    ), 0)
                    _q(nc.gpsimd.indirect_dma_start(
                        out=gr[:, c, :], out_offset=None, in_=table[:],
                        in_offset=bass.IndirectOffsetOnAxis(ap=r_t[:, c:c + 1], axis=0),
                    ), 0)
                return s_t, r_t, ef_t, gs, gr

            def stage_b(ch, ctx):
                s_t, r_t, ef_t, gs, gr = ctx
                pse = ps_e.tile([128, 16, EMBED], F32, space="PSUM", tag="ep")
                for c in range(16):
                    nc.tensor.matmul(
                        out=pse[:, c, :], lhsT=ef_t[:, c * 128:(c + 1) * 128],
                        rhs=We_t[:], start=True, stop=True)
                x_t = bpool.tile([128, 16, EMBED], F32, tag="x")
                xf = x_t[:].rearrange("p c d -> p (c d)")
                nc.vector.tensor_add(
                    xf, gs[:].rearrange("p c d -> p (c d)"),
                    gr[:].rearrange("p c d -> p (c d)"))
                nc.vector.tensor_add(xf, xf, pse[:].rearrange("p c d -> p (c d)"))
                nc.vector.tensor_add(x_t[:], x_t[:], _ap3(Web_t[:], 16))
                # mish(x) = x * (u^2+2u)/(u^2+2u+2), u=e^x (exp-only, exact)
                u_t = bpool.tile([128, 16 * EMBED], F32, tag="mu")
                tb_t = bpool.tile([128, 16 * EMBED], F32, tag="mtb")
                nc.scalar.activation(u_t[:], xf, mybir.ActivationFunctionType.Exp)
                nc.vector.tensor_scalar_add(tb_t[:], u_t[:], 2.0)
                nc.vector.tensor_mul(u_t[:], u_t[:], tb_t[:])
                nc.vector.tensor_scalar_add(tb_t[:], u_t[:], 2.0)
                nc.vector.reciprocal(tb_t[:], tb_t[:])
                nc.vector.tensor_mul(u_t[:], u_t[:], tb_t[:])
                xm_t = bpool.tile([128, 16, EMBED], F32, tag="xm")
                nc.vector.tensor_mul(
                    xm_t[:].rearrange("p c d -> p (c d)"), xf, u_t[:])
                lg_t = bpool.tile([128, 16 * EMBED], F32, tag="lgm")
                nc.vector.tensor_mul(
                    lg_t[:], xm_t[:].rearrange("p c d -> p (c d)"), a_t[:])
                l_t = bpool.tile([128, 16 * HEADS], F32, tag="lg")
                nc.vector.tensor_reduce(
                    l_t[:].rearrange("p (g o) -> p g o", o=1),
                    lg_t[:].rearrange("p (g i) -> p g i", i=8),
                    axis=mybir.AxisListType.X, op=mybir.AluOpType.add)
                w_t = bpool.tile([128, 16 * HEADS], F32, tag="w")
                nc.scalar.activation(
                    w_t[:], l_t[:], mybir.ActivationFunctionType.Exp)
                pay_t = bpool.tile([128, 16, PAY], F32, tag="pay")
                nc.vector.tensor_mul(
                    pay_t[:, :, :EMBED].rearrange("p c (h o) -> p c h o", o=8),
                    gs[:].rearrange("p c (h o) -> p c h o", o=8),
                    _inner_b(w_t[:].rearrange("p (c h) -> p c h", h=8), 8))
                nc.vector.tensor_copy(
                    pay_t[:, :, EMBED:], w_t[:].rearrange("p (c h) -> p c h", h=8))

                rf_t = spool.tile([128, 16], F32, tag="rf")
                nc.vector.tensor_copy(rf_t[:], r_t[:])
                for hb in range(2):
                    cset = list(range(hb * 8, hb * 8 + 8))
                    pst = ps_t.tile([128, 8, 128], F32, space="PSUM", tag="tr")
                    for j, c in enumerate(cset):
                        nc.tensor.transpose(
                            out=pst[:, j, :],
                            in_=rf_t[:, c:c + 1].to_broadcast([128, 128]),
                            identity=idt[:])
                    tsame = spool.tile([128, 8, 128], F32, tag="tsame")
                    nc.vector.tensor_tensor(
                        out=tsame[:],
                        in0=_inner_b(rf_t[:, hb * 8:hb * 8 + 8], 128),
                        in1=pst[:],
                        op=mybir.AluOpType.is_equal)
                    cmask = spool.tile([128, 8, 128], F32, tag="cmask")
                    nc.vector.tensor_mul(cmask[:], tsame[:], _ap3(lm_t[:], 8))
                    psr = ps_r.tile([128, 8], F32, space="PSUM", tag="rk")
                    psp = ps_p.tile([128, 8, 128], F32, space="PSUM", tag="pr")
                    for j, c in enumerate(cset):
                        nc.tensor.matmul(out=psr[:, j:j + 1], lhsT=cmask[:, j, :],
                                         rhs=ones_t[:], start=True, stop=True)
                        nc.tensor.matmul(out=psp[:, j, :PAY], lhsT=tsame[:, j, :],
                                         rhs=pay_t[:, c, :], start=True, stop=True)
                    occ = spool.tile([128, 8], F32, tag="occ")
                    nc.vector.tensor_tensor(out=occ[:], in0=psr[:],
                                            in1=zero1_t[:, :8],
                                            op=mybir.AluOpType.is_equal)
                    self_f = spool.tile([128, 8], F32, tag="self")
                    nc.vector.tensor_scalar_add(
                        self_f[:], rf_t[:, hb * 8:hb * 8 + 8], float(-TRASH))
                    nc.vector.tensor_mul(self_f[:], self_f[:], occ[:])
                    nc.vector.tensor_scalar_add(self_f[:], self_f[:], float(TRASH))
                    sidx = spool.tile([128, 8], I32, tag="scix")
                    nc.vector.tensor_copy(sidx[:], self_f[:])
                    pres = spool.tile([128, 8, PAY], F32, tag="pres")
                    nc.vector.tensor_copy(pres[:], psp[:, :, :PAY])
                    for j, c in enumerate(cset):
                        gcol = ch * 16 + c
                        _q(nc.gpsimd.indirect_dma_start(
                            out=accs[gcol % NT][:],
                            out_offset=bass.IndirectOffsetOnAxis(
                                ap=sidx[:, j:j + 1], axis=0),
                            in_=pres[:, j, :],
                            in_offset=None,
                            compute_op=mybir.AluOpType.add,
                        ), 0)

            pend = {}
            for ch in range(nch):
                pend[ch] = stage_a(ch)
                if ch - LOOKAHEAD >= 0:
                    stage_b(ch - LOOKAHEAD, pend.pop(ch - LOOKAHEAD))
            for ch in sorted(pend):
                stage_b(ch, pend.pop(ch))

            # ---- phase 3: merge NT tables -> merged
            MR = 14  # rows per partition per group; 392/14 = 28 groups
            for g in range(NPAD // (128 * MR)):
                r0 = g * 128 * MR
                mt = mpool.tile([128, MR, PAY], F32, tag="mg")
                nc.sync.dma_start(
                    out=mt[:],
                    in_=accs[0][r0:r0 + 128 * MR, :].rearrange(
                        "(p c) d -> p c d", p=128))
                for t in range(1, NT):
                    at = mpool.tile([128, MR, PAY], F32, tag="ma")
                    nc.sync.dma_start(
                        out=at[:],
                        in_=accs[t][r0:r0 + 128 * MR, :].rearrange(
                            "(p c) d -> p c d", p=128))
                    nc.vector.tensor_add(
                        mt[:].rearrange("p c d -> p (c d)"),
                        mt[:].rearrange("p c d -> p (c d)"),
                        at[:].rearrange("p c d -> p (c d)"))
                nc.sync.dma_start(
                    out=merged[r0:r0 + 128 * MR, :].rearrange(
                        "(p c) d -> p c d", p=128),
                    in_=mt[:])

            # ---- phase 4: ReduceScatter + divide + out
            nc.gpsimd.collective_compute(
                "ReduceScatter",
                mybir.AluOpType.add,
                replica_groups=[list(range(N_CORES))],
                ins=[merged[:]],
                outs=[rs_out[:]])
            fin = fpool.tile([128, RSC, PAY], F32, tag="fin")
            nc.sync.dma_start(
                out=fin[:], in_=rs_out[:].rearrange("(p c) d -> p c d", p=128))
            den = fpool.tile([128, RSC, HEADS], F32, tag="den")
            nc.vector.tensor_scalar_add(den[:], fin[:, :, EMBED:], 1e-30)
            rec = fpool.tile([128, RSC, HEADS], F32, tag="rec")
            nc.vector.reciprocal(rec[:], den[:])
            ot = fpool.tile([128, RSC, EMBED], F32, tag="ot")
            nc.vector.tensor_mul(
                ot[:].rearrange("p c (h o) -> p c h o", o=8),
                fin[:, :, :EMBED].rearrange("p c (h o) -> p c h o", o=8),
                _inner_b(rec[:], 8))
            nc.sync.dma_start(out=out_e[:], in_=ot[:])

    _split_excess_waits(nc)
    return nc


def host_prep(node_features, edge_features, senders, receivers,
              W_kernel, W_bias, We_kernel, We_bias, a,
              n_cores=N_CORES, nch=NCH):
    """Pure layout transforms -> per-core input maps."""
    slots = nch * CHUNK
    cols = slots // 128
    epc = min(EPC, slots)

    nf_pad = np.zeros((NPAD, IN_DIM), np.float32)
    nf_pad[:N_NODES] = node_features
    nfT = np.ascontiguousarray(nf_pad.T)

    Wb_rep = np.tile(np.asarray(W_bias, np.float32)[None, :], (128, 1))
    Web_rep = np.tile(np.asarray(We_bias, np.float32)[None, :], (128, 1))
    a16 = np.tile(np.asarray(a, np.float32).reshape(-1)[None, :], (128, 16))
    lmask = np.triu(np.ones((128, 128), np.float32), 1)

    in_maps = []
    for c in range(n_cores):
        lo = c * epc
        hi = min(lo + epc, len(senders))
        n_real = hi - lo
        s_pad = np.full(slots, PADNODE, np.int32)
        r_pad = np.full(slots, PADNODE, np.int32)
        ef_pad = np.zeros((slots, EDGE_DIM), np.float32)
        s_pad[:n_real] = senders[lo:hi]
        r_pad[:n_real] = receivers[lo:hi]
        ef_pad[:n_real] = edge_features[lo:hi]
        in_maps.append({
            "nfT": nfT,
            "eftT": np.ascontiguousarray(ef_pad.T),
            "s_wrap": np.ascontiguousarray(s_pad.reshape(cols, 128).T),
            "r_wrap": np.ascontiguousarray(r_pad.reshape(cols, 128).T),
            "W": np.asarray(W_kernel, np.float32),
            "Wb": Wb_rep,
            "We": np.asarray(We_kernel, np.float32),
            "Web": Web_rep,
            "a16": a16,
            "lmask": lmask,
        })
    return in_maps


def _build_runner(nc, n_cores):
    """Jitted 8-core SPMD executor via the axon PJRT tunnel (shard_map)."""
    import time
    import jax
    from jax.sharding import Mesh, PartitionSpec
    from jax.experimental.shard_map import shard_map
    from concourse import bass2jax
    from concourse.bass2jax import _bass_exec_p, install_neuronx_cc_hook

    install_neuronx_cc_hook()
    partition_name = nc.partition_id_tensor.name if nc.partition_id_tensor else None
    in_names, out_names, out_avals, zero_outs = [], [], [], []
    for alloc in nc.m.functions[0].allocations:
        if not isinstance(alloc, mybir.MemoryLocationSet):
            continue
        name = alloc.memorylocations[0].name
        if alloc.kind == "ExternalInput":
            if name != partition_name:
                in_names.append(name)
        elif alloc.kind == "ExternalOutput":
            out_names.append(name)
            shape = tuple(alloc.tensor_shape)
            dtype = mybir.dt.np(alloc.dtype)
            out_avals.append(jax.core.ShapedArray(shape, dtype))
            zero_outs.append(np.zeros(shape, dtype))
    n_params = len(in_names)
    n_outs = len(out_avals)
    all_in_names = list(in_names) + list(out_names)
    if partition_name is not None:
        all_in_names.append(partition_name)

    def _body(*args):
        operands = list(args)
        if partition_name is not None:
            operands.append(bass2jax.partition_id_tensor())
        return tuple(_bass_exec_p.bind(
            *operands,
            out_avals=tuple(out_avals),
            in_names=tuple(all_in_names),
            out_names=tuple(out_names),
            lowering_input_output_aliases=(),
            sim_require_finite=True,
            sim_require_nnan=True,
            nc=nc,
        ))

    donate = tuple(range(n_params, n_params + n_outs))
    devices = jax.devices()[:n_cores]
    mesh = Mesh(np.asarray(devices), ("core",))
    in_specs = (PartitionSpec("core"),) * (n_params + n_outs)
    out_specs = (PartitionSpec("core"),) * len(out_names)
    jfn = jax.jit(
        shard_map(_body, mesh=mesh, in_specs=in_specs, out_specs=out_specs,
                  check_rep=False),
        donate_argnums=donate, keep_unused=True)

    def fn(in_maps):
        concat_in = [
            np.concatenate([np.asarray(in_maps[c][n]) for c in range(n_cores)], 0)
            for n in in_names
        ]
        concat_zeros = [np.zeros((n_cores * z.shape[0], *z.shape[1:]), z.dtype)
                        for z in zero_outs]
        t0 = time.perf_counter()
        out_arrs = jfn(*concat_in, *concat_zeros)
        out_arrs = [np.asarray(o) for o in out_arrs]
        dt = time.perf_counter() - t0
        return [
            {n: out_arrs[i].reshape(n_cores, *out_avals[i].shape)[c]
             for i, n in enumerate(out_names)}
            for c in range(n_cores)
        ], dt

    return fn


_CACHE = {}


def kernel(node_features, edge_features, global_features, senders, receivers,
           W_kernel, W_bias, We_kernel, We_bias, a):
    node_features = np.asarray(node_features, np.float32)
    edge_features = np.asarray(edge_features, np.float32)
    senders = np.asarray(senders, np.int32)
    receivers = np.asarray(receivers, np.int32)
    in_maps = host_prep(node_features, edge_features, senders, receivers,
                        W_kernel, W_bias, We_kernel, We_bias, a)
    if "fn" not in _CACHE:
        nc = build_nc()
        _CACHE["fn"] = _build_runner(nc, N_CORES)
    res, dt = _CACHE["fn"](in_maps)
    _CACHE["last_dt"] = dt
    # out_shard [128, RSC, 64]; row index within shard = p*RSC + c
    full = np.concatenate(
        [r["out_shard"].reshape(128 * RSC, EMBED) for r in res], axis=0)
    return full[:N_NODES].astype(np.float32)



# revision 2
# speedup vs baseline: 1.4812x; 1.4812x over previous
"""GATv2 message-passing kernel for 8 Trainium2 NeuronCores (Bass/Tile), v2.

Strategy: shard by RECEIVER RANGE. Host sorts edges by receiver; core c owns
output nodes [c*6272, (c+1)*6272) and the edges pointing at them. Each
receiver-tile of 128 nodes gets a static edge-slot capacity C = G*128 (G from
data, padded slots masked). Per core:
  phase 1:  project all nodes -> DRAM table [NPAD, 64] (replicated compute)
  phase 1b: project own node range -> SBUF-resident rtab [128, 49, 64]
  phase 2:  per chunk (half receiver-tile, ng groups of 128 edge slots):
    - dense edge-feature load (transposed, ones-row folds We_bias)
    - ng batched send-row indirect gathers (queue-rotated over 4 SWDGE qs)
    - one-hot R [slot, j] built on DVE (is_equal vs iota); PE-transposed to
      R_T, ACT-copied to SBUF
    - PE: x_psum = eft^T @ We_ext + R_T^T @ rtab_tile  (edge proj + recv row
      expansion accumulated in PSUM); DVE adds gathered send rows
    - ACT Mish, DVE logits, ACT Exp -> w, mask; payload [w*send ; w]
    - PE: psn += R^T @ payload accumulates the ENTIRE segment softmax sum
      [128 nodes, 72] (no scatters, no collectives)
  divide, store output shard directly. Host concatenates the 8 shards.
"""
import sys

sys.path.insert(0, "/opt/trn_rl_repo")
import numpy as np
import concourse.bass as bass
import concourse.mybir as mybir
import concourse.tile as tile
import concourse.tile as tile_mod
from concourse.masks import make_identity
from concourse.vector_clock import ScopedClock

# --- walrus build workarounds (same as baseline kernel.py) ---
try:
    from jax.interpreters import mlir as _mlir
    from concourse.bass2jax import (
        _bass_exec_p as _bep,
        _bass_exec_neuron_lowering as _benl,
        _partition_id_p as _pip,
        _partition_id_lowering as _pil,
    )

    _mlir.register_lowering(_bep, _benl, platform="axon")
    _mlir.register_lowering(_pip, _pil, platform="axon")
except Exception:  # pragma: no cover
    pass

_N_CARRIERS = 24


def _patched_drain_and_barrier(self, tick_clock, wait_clock):
    nc = self.nc
    nops = [nc.sync.nop(nofuse=True) for _ in range(_N_CARRIERS)]
    drain_inst = nc.sync.drain()
    wait_clock.add_sem_waits(
        drain_inst.ins, ScopedClock({None: tick_clock.global_clock}))
    waits = list(drain_inst.ins.sync_info.on_wait or [])
    if len(waits) > 1:
        assert len(waits) - 1 <= _N_CARRIERS
        drain_inst.ins.sync_info.on_wait = waits[:1]
        for nop, w in zip(nops, waits[1:]):
            si = nop.ins.sync_info
            if si is None:
                nop.ins.sync_info = mybir.SyncInfo(on_wait=[w], on_update=[])
            else:
                si.on_wait = [w]
    nc.all_engine_barrier()
    assert self.sems is not None
    popped = nc._tile_sem_poison_stack.pop()
    assert popped is self._sem_poison
    nc.clear_and_free_semaphores(list(self.sems.allocated().values()))
    nc.all_engine_barrier()


tile_mod.TileContext._drain_and_barrier = _patched_drain_and_barrier


def _split_excess_waits(nc, max_waits=1):
    for bbname, body in nc.bb_map.items():
        bb = body.bb
        insts = list(bb.instructions)
        out = []
        changed = False
        for ins in insts:
            si = ins.sync_info
            waits = list(si.on_wait) if si and si.on_wait else []
            if len(waits) > max_waits:
                keep = waits[:max_waits - 1] + [waits[-1]]
                extra = waits[max_waits - 1:-1]
                for w in extra:
                    nop = mybir.InstNoOp(
                        name=nc.get_next_instruction_name(), ins=[], outs=[])
                    nop.engine = ins.engine
                    nop.sync_info = mybir.SyncInfo(on_wait=[w], on_update=[])
                    nc.register_instruction(nop, overwrite=True)
                    out.append(nop)
                ins.sync_info.on_wait = keep
                changed = True
            out.append(ins)
        if changed:
            bb.instructions = out


F32 = mybir.dt.float32
I32 = mybir.dt.int32

N_NODES = 50000
N_EDGES = 1200000
IN_DIM = 128
EDGE_DIM = 64
EMBED = 64
HEADS = 8
PAY = EMBED + HEADS  # 72

N_CORES = 8
NPC = 6272            # nodes per core = 49 tiles of 128
NTILE = NPC // 128    # 49
NPAD = NPC * N_CORES  # 50176
MR = 4                # node-tiles per phase-1 iteration
MR2 = 7               # node-tiles per phase-1b iteration
NQ = 4                # SWDGE queues for gather rotation


def _ap3(ap, mid_n):
    """[128, D] AP -> [128, mid_n(step0), D] broadcast view."""
    return bass.AP(ap.tensor, ap.offset, [ap.ap[0], [0, mid_n]] + list(ap.ap[1:]))


def _inner_b(ap, n):
    """Append a step-0 innermost free dim of size n (broadcast view)."""
    return bass.AP(ap.tensor, ap.offset, list(ap.ap) + [[0, n]])


def build_nc(G, reps=1, use_mish=False):
    """G = edge-slot groups (of 128) per receiver tile (even)."""
    assert G % 2 == 0
    C = G * 128
    SUB = G // 2  # groups per chunk; 2 chunks per receiver tile
    assert SUB >= MR2, "phase-1 PSUM tiles alias the phase-2 'px' ring"
    nc = bass.Bass(num_swdge_queues=NQ)

    nfT = nc.declare_dram_parameter("nfT", [IN_DIM, NPAD], F32, isOutput=False)
    rnfT = nc.declare_dram_parameter("rnfT", [IN_DIM, NPC], F32, isOutput=False)
    eftT = nc.declare_dram_parameter("eftT", [EDGE_DIM + 1, NTILE * C], F32,
                                     isOutput=False)
    si_e = nc.declare_dram_parameter("s_idx", [128, NTILE * G], I32, isOutput=False)
    rf_e = nc.declare_dram_parameter("r_rel", [128, NTILE * G], F32, isOutput=False)
    mk_e = nc.declare_dram_parameter("mask", [128, NTILE * G], F32, isOutput=False)
    W_e = nc.declare_dram_parameter("W", [IN_DIM, EMBED], F32, isOutput=False)
    Wb_e = nc.declare_dram_parameter("Wb", [128, EMBED], F32, isOutput=False)
    We_e = nc.declare_dram_parameter("We_ext", [EDGE_DIM + 1, EMBED], F32,
                                     isOutput=False)
    a_e = nc.declare_dram_parameter("a64", [128, EMBED], F32, isOutput=False)
    io_e = nc.declare_dram_parameter("iota", [128, 128], F32, isOutput=False)
    out_e = nc.declare_dram_parameter("out_shard", [NPC, EMBED], F32, isOutput=True)

    table = nc.dram_tensor("ntable", [NPAD, EMBED], F32)

    with tile.TileContext(nc) as tc:
        with (
            tc.tile_pool(name="const", bufs=1) as cpool,
            tc.tile_pool(name="meta", bufs=1) as mpool,
            tc.tile_pool(name="nproj", bufs=3) as npool,
            tc.tile_pool(name="edgea", bufs=3) as apool,
            tc.tile_pool(name="edgeb", bufs=2) as bpool,
            tc.tile_pool(name="small", bufs=2) as spool,
            tc.tile_pool(name="fin", bufs=2) as fpool,
            tc.tile_pool(name="ps_x", bufs=2, space="PSUM") as ps_x,
            tc.tile_pool(name="ps_n", bufs=2, space="PSUM") as ps_n,
            tc.tile_pool(name="ps_t", bufs=2, space="PSUM") as ps_t,
        ):
            # ---- constants & resident metadata
            W_t = cpool.tile([IN_DIM, EMBED], F32)
            nc.sync.dma_start(out=W_t[:], in_=W_e[:])
            Wb_t = cpool.tile([128, EMBED], F32)
            nc.sync.dma_start(out=Wb_t[:], in_=Wb_e[:])
            We_t = cpool.tile([EDGE_DIM + 1, EMBED], F32)
            nc.sync.dma_start(out=We_t[:], in_=We_e[:])
            a_t = cpool.tile([128, EMBED], F32)
            nc.sync.dma_start(out=a_t[:], in_=a_e[:])
            io_t = cpool.tile([128, 128], F32)
            nc.sync.dma_start(out=io_t[:], in_=io_e[:])
            idt = cpool.tile([128, 128], F32)
            make_identity(nc, idt[:])
            si_t = mpool.tile([128, NTILE * G], I32)
            nc.sync.dma_start(out=si_t[:], in_=si_e[:])
            rf_t = mpool.tile([128, NTILE * G], F32)
            nc.sync.dma_start(out=rf_t[:], in_=rf_e[:])
            mk_t = mpool.tile([128, NTILE * G], F32)
            nc.sync.dma_start(out=mk_t[:], in_=mk_e[:])
            rtab = mpool.tile([128, NTILE, EMBED], F32)  # own-range projections

            for _ in range(reps):
                # ---- phase 1: node projection -> table (replicated)
                for t in range(NPAD // (128 * MR)):
                    nf_t = npool.tile([IN_DIM, 128 * MR], F32, tag="nf")
                    nc.sync.dma_start(
                        out=nf_t[:],
                        in_=nfT[:, t * 128 * MR:(t + 1) * 128 * MR])
                    ps = ps_x.tile([128, SUB, EMBED], F32, space="PSUM", tag="px")
                    for g in range(MR):
                        nc.tensor.matmul(
                            out=ps[:, g, :],
                            lhsT=nf_t[:, g * 128:(g + 1) * 128],
                            rhs=W_t[:], start=True, stop=True)
                    nb = npool.tile([128, MR, EMBED], F32, tag="nb")
                    nc.vector.tensor_add(nb[:], ps[:, :MR, :], _ap3(Wb_t[:], MR))
                    nc.sync.dma_start(
                        out=table[t * 128 * MR:(t + 1) * 128 * MR, :].rearrange(
                            "(c p) d -> p c d", p=128),
                        in_=nb[:])

                # ---- phase 1b: own-range projection -> SBUF rtab
                for t in range(NTILE // MR2):
                    nf_t = npool.tile([IN_DIM, 128 * MR2], F32, tag="nf2")
                    nc.sync.dma_start(
                        out=nf_t[:],
                        in_=rnfT[:, t * 128 * MR2:(t + 1) * 128 * MR2])
                    ps = ps_x.tile([128, SUB, EMBED], F32, space="PSUM", tag="px")
                    for g in range(MR2):
                        nc.tensor.matmul(
                            out=ps[:, g, :],
                            lhsT=nf_t[:, g * 128:(g + 1) * 128],
                            rhs=W_t[:], start=True, stop=True)
                    nc.vector.tensor_add(
                        rtab[:, t * MR2:(t + 1) * MR2, :], ps[:, :MR2, :],
                        _ap3(Wb_t[:], MR2))

                # ---- phase 2: receiver tiles (2 chunks of SUB groups each)
                for t in range(NTILE):
                    psn = ps_n.tile([128, PAY], F32, space="PSUM", tag="pn")
                    for hi in range(2):
                        ng = SUB
                        g0 = t * G + hi * SUB
                        c0 = t * C + hi * SUB * 128
                        eft_t = apool.tile([EDGE_DIM + 1, ng * 128], F32,
                                           tag="eft")
                        nc.sync.dma_start(
                            out=eft_t[:], in_=eftT[:, c0:c0 + ng * 128])
                        gs = apool.tile([128, ng, EMBED], F32, tag="gs")
                        for g in range(ng):
                            bi = nc.gpsimd.indirect_dma_start(
                                out=gs[:, g, :], out_offset=None, in_=table[:],
                                in_offset=bass.IndirectOffsetOnAxis(
                                    ap=si_t[:, g0 + g:g0 + g + 1], axis=0))
                            q = (t * 2 * ng + hi * ng + g) % NQ
                            if q:
                                bi.ins.queue = f"qPoolDynamic{q}"
                        # one-hot R over receiver-relative idx
                        R_t = spool.tile([128, ng, 128], F32, tag="R")
                        nc.vector.tensor_tensor(
                            out=R_t[:],
                            in0=_inner_b(rf_t[:, g0:g0 + ng], 128),
                            in1=_ap3(io_t[:], ng),
                            op=mybir.AluOpType.is_equal)
                        RT_t = spool.tile([128, ng, 128], F32, tag="RT")
                        for g in range(ng):
                            pst = ps_t.tile([128, 128], F32, space="PSUM",
                                            tag="tr")
                            nc.tensor.transpose(
                                out=pst[:], in_=R_t[:, g, :], identity=idt[:])
                            nc.scalar.activation(
                                RT_t[:, g, :], pst[:],
                                mybir.ActivationFunctionType.Copy)
                        # x_psum = edge projection + recv-row expansion
                        psx = ps_x.tile([128, ng, EMBED], F32, space="PSUM",
                                        tag="px")
                        for g in range(ng):
                            nc.tensor.matmul(
                                out=psx[:, g, :],
                                lhsT=eft_t[:, g * 128:(g + 1) * 128],
                                rhs=We_t[:], start=True, stop=False)
                            nc.tensor.matmul(
                                out=psx[:, g, :],
                                lhsT=RT_t[:, g, :],
                                rhs=rtab[:, t, :], start=False, stop=True)
                        x_t = bpool.tile([128, ng, EMBED], F32, tag="x")
                        xf = x_t[:].rearrange("p c d -> p (c d)")
                        nc.vector.tensor_add(
                            xf, gs[:].rearrange("p c d -> p (c d)"),
                            psx[:].rearrange("p c d -> p (c d)"))
                        xm_t = bpool.tile([128, ng, EMBED], F32, tag="xm")
                        xmf = xm_t[:].rearrange("p c d -> p (c d)")
                        if use_mish:
                            nc.scalar.activation(
                                xmf, xf, mybir.ActivationFunctionType.Mish)
                        else:
                            # mish(x) = x*(u^2+2u)/(u^2+2u+2), u=e^x
                            u_t = bpool.tile([128, ng * EMBED], F32, tag="mu")
                            tb_t = bpool.tile([128, ng * EMBED], F32, tag="mtb")
                            nc.scalar.activation(
                                u_t[:], xf, mybir.ActivationFunctionType.Exp)
                            nc.vector.tensor_scalar_add(tb_t[:], u_t[:], 2.0)
                            nc.vector.tensor_mul(u_t[:], u_t[:], tb_t[:])
                            nc.vector.tensor_scalar_add(tb_t[:], u_t[:], 2.0)
                            nc.vector.reciprocal(tb_t[:], tb_t[:])
                            nc.vector.tensor_mul(u_t[:], u_t[:], tb_t[:])
                            nc.vector.tensor_mul(xmf, xf, u_t[:])
                        lg_t = bpool.tile([128, ng * EMBED], F32, tag="lg")
                        nc.vector.tensor_mul(
                            lg_t[:].rearrange("p (c d) -> p c d", d=EMBED),
                            xm_t[:], _ap3(a_t[:], ng))
                        l_t = spool.tile([128, ng * HEADS], F32, tag="l")
                        nc.vector.tensor_reduce(
                            l_t[:].rearrange("p (q o) -> p q o", o=1),
                            lg_t[:].rearrange("p (q i) -> p q i", i=8),
                            axis=mybir.AxisListType.X, op=mybir.AluOpType.add)
                        pay_t = bpool.tile([128, ng, PAY], F32, tag="pay")
                        wv = pay_t[:, :, EMBED:]  # [128, ng, 8]
                        nc.scalar.activation(
                            wv, l_t[:].rearrange("p (c h) -> p c h", h=HEADS),
                            mybir.ActivationFunctionType.Exp)
                        nc.vector.tensor_mul(
                            wv, wv, _inner_b(mk_t[:, g0:g0 + ng], HEADS))
                        nc.vector.tensor_mul(
                            pay_t[:, :, :EMBED].rearrange(
                                "p c (h o) -> p c h o", o=8),
                            gs[:].rearrange("p c (h o) -> p c h o", o=8),
                            _inner_b(wv, 8))
                        for g in range(ng):
                            nc.tensor.matmul(
                                out=psn[:], lhsT=R_t[:, g, :],
                                rhs=pay_t[:, g, :],
                                start=(hi == 0 and g == 0),
                                stop=(hi == 1 and g == ng - 1))
                    den = fpool.tile([128, HEADS], F32, tag="den")
                    nc.vector.tensor_scalar_add(den[:], psn[:, EMBED:], 1e-30)
                    rec = fpool.tile([128, HEADS], F32, tag="rec")
                    nc.vector.reciprocal(rec[:], den[:])
                    ot = fpool.tile([128, EMBED], F32, tag="ot")
                    nc.vector.tensor_mul(
                        ot[:].rearrange("p (h o) -> p h o", o=8),
                        psn[:, :EMBED].rearrange("p (h o) -> p h o", o=8),
                        _inner_b(rec[:], 8))
                    nc.sync.dma_start(
                        out=out_e[t * 128:(t + 1) * 128, :], in_=ot[:])

    _split_excess_waits(nc)
    return nc


def host_prep(node_features, edge_features, senders, receivers,
              W_kernel, W_bias, We_kernel, We_bias, a):
    """Sort edges by receiver, build per-core tile-padded streams."""
    node_features = np.asarray(node_features, np.float32)
    edge_features = np.asarray(edge_features, np.float32)
    senders = np.asarray(senders, np.int32)
    receivers = np.asarray(receivers, np.int32)

    order = np.argsort(receivers, kind="stable")
    s_s = senders[order]
    r_s = receivers[order]
    ef_s = edge_features[order]

    gtile = r_s // 128  # global receiver tile 0..391
    n_gtiles = NPAD // 128  # 392
    counts = np.bincount(gtile, minlength=n_gtiles)
    G = max(2, int(np.ceil(counts.max() / 128)))
    if G % 2:
        G += 1
    C = G * 128

    starts = np.zeros(n_gtiles, np.int64)
    starts[1:] = np.cumsum(counts)[:-1]
    within = np.arange(len(r_s)) - starts[gtile]
    slot = gtile * C + within

    tot_slots = n_gtiles * C
    s_pad = np.zeros(tot_slots, np.int32)
    rrel_pad = np.zeros(tot_slots, np.float32)
    mask_pad = np.zeros(tot_slots, np.float32)
    ef_pad = np.zeros((tot_slots, EDGE_DIM + 1), np.float32)
    s_pad[slot] = s_s
    rrel_pad[slot] = (r_s % 128).astype(np.float32)
    mask_pad[slot] = 1.0
    ef_pad[slot, :EDGE_DIM] = ef_s
    ef_pad[:, EDGE_DIM] = 1.0  # ones-row for folded We_bias

    nf_pad = np.zeros((NPAD, IN_DIM), np.float32)
    nf_pad[:N_NODES] = node_features
    nfT = np.ascontiguousarray(nf_pad.T)

    Wb_rep = np.tile(np.asarray(W_bias, np.float32)[None, :], (128, 1))
    We_ext = np.concatenate(
        [np.asarray(We_kernel, np.float32),
         np.asarray(We_bias, np.float32)[None, :]], axis=0)
    a64 = np.tile(np.asarray(a, np.float32).reshape(-1)[None, :], (128, 1))
    iota = np.tile(np.arange(128, dtype=np.float32)[None, :], (128, 1))

    def wrap_idx(arr):
        # [49*C] -> [128, NTILE*G]: col t*G+g, partition p <- slot t*C+g*128+p
        return np.ascontiguousarray(
            arr.reshape(NTILE, G, 128).transpose(2, 0, 1).reshape(128, NTILE * G))

    in_maps = []
    for c in range(N_CORES):
        lo, hi = c * NTILE * C, (c + 1) * NTILE * C
        in_maps.append({
            "nfT": nfT,
            "rnfT": np.ascontiguousarray(nf_pad[c * NPC:(c + 1) * NPC].T),
            "eftT": np.ascontiguousarray(ef_pad[lo:hi].T),
            "s_idx": wrap_idx(s_pad[lo:hi]),
            "r_rel": wrap_idx(rrel_pad[lo:hi]),
            "mask": wrap_idx(mask_pad[lo:hi]),
            "W": np.asarray(W_kernel, np.float32),
            "Wb": Wb_rep,
            "We_ext": We_ext,
            "a64": a64,
            "iota": iota,
        })
    return in_maps, G


def _build_runner(nc, n_cores):
    import time
    import jax
    from jax.sharding import Mesh, PartitionSpec
    from jax.experimental.shard_map import shard_map
    from concourse import bass2jax
    from concourse.bass2jax import _bass_exec_p, install_neuronx_cc_hook

    install_neuronx_cc_hook()
    partition_name = nc.partition_id_tensor.name if nc.partition_id_tensor else None
    in_names, out_names, out_avals, zero_outs = [], [], [], []
    for alloc in nc.m.functions[0].allocations:
        if not isinstance(alloc, mybir.MemoryLocationSet):
            continue
        name = alloc.memorylocations[0].name
        if alloc.kind == "ExternalInput":
            if name != partition_name:
                in_names.append(name)
        elif alloc.kind == "ExternalOutput":
            out_names.append(name)
            shape = tuple(alloc.tensor_shape)
            dtype = mybir.dt.np(alloc.dtype)
            out_avals.append(jax.core.ShapedArray(shape, dtype))
            zero_outs.append(np.zeros(shape, dtype))
    n_params = len(in_names)
    n_outs = len(out_avals)
    all_in_names = list(in_names) + list(out_names)
    if partition_name is not None:
        all_in_names.append(partition_name)

    def _body(*args):
        operands = list(args)
        if partition_name is not None:
            operands.append(bass2jax.partition_id_tensor())
        return tuple(_bass_exec_p.bind(
            *operands,
            out_avals=tuple(out_avals),
            in_names=tuple(all_in_names),
            out_names=tuple(out_names),
            lowering_input_output_aliases=(),
            sim_require_finite=True,
            sim_require_nnan=True,
            nc=nc,
        ))

    donate = tuple(range(n_params, n_params + n_outs))
    devices = jax.devices()[:n_cores]
    mesh = Mesh(np.asarray(devices), ("core",))
    in_specs = (PartitionSpec("core"),) * (n_params + n_outs)
    out_specs = (PartitionSpec("core"),) * len(out_names)
    jfn = jax.jit(
        shard_map(_body, mesh=mesh, in_specs=in_specs, out_specs=out_specs,
                  check_rep=False),
        donate_argnums=donate, keep_unused=True)

    def fn(in_maps):
        concat_in = [
            np.concatenate([np.asarray(in_maps[c][n]) for c in range(n_cores)], 0)
            for n in in_names
        ]
        concat_zeros = [np.zeros((n_cores * z.shape[0], *z.shape[1:]), z.dtype)
                        for z in zero_outs]
        t0 = time.perf_counter()
        out_arrs = jfn(*concat_in, *concat_zeros)
        out_arrs = [np.asarray(o) for o in out_arrs]
        dt = time.perf_counter() - t0
        return [
            {n: out_arrs[i].reshape(n_cores, *out_avals[i].shape)[c]
             for i, n in enumerate(out_names)}
            for c in range(n_cores)
        ], dt

    return fn


_CACHE = {}


def kernel(node_features, edge_features, global_features, senders, receivers,
           W_kernel, W_bias, We_kernel, We_bias, a):
    in_maps, G = host_prep(node_features, edge_features, senders, receivers,
                           W_kernel, W_bias, We_kernel, We_bias, a)
    if _CACHE.get("G") != G:
        nc = build_nc(G)
        _CACHE["fn"] = _build_runner(nc, N_CORES)
        _CACHE["G"] = G
    res, dt = _CACHE["fn"](in_maps)
    _CACHE["last_dt"] = dt
    full = np.concatenate([r["out_shard"] for r in res], axis=0)
    return full[:N_NODES].astype(np.float32)
